# revision 2
# baseline (speedup 1.0000x reference)
"""DeepSeekV2 MoE (E=8, top-2, H=2048, F=1408, T=2048) on 8 TRN2 NeuronCores.

Strategy (expert-parallel, dense):
  - Host: replicate x (transposed layouts) to all cores; shard the 8 experts'
    weight stacks one expert per core; all weights/activations for the big
    GEMMs are cast to fp16 (router stays fp32).
  - Each core c:
      router:  logits = x @ gate_w in fp32 on PE, top-2 + renormalized
               combine weights computed on-device (replicated); core c keeps
               its own expert's column via a one-hot input.
      expert:  G = wg_c.T-layout GEMM, U likewise -> A = silu(G)*U  [F, T] fp16
               Y = A.T @ wd_c  [T, H], scaled rows by combine_c      (all PE fp16)
      combine: scaled Y (fp16) AllToAll'd over the token dim; each core sums
               the 8 received partials for its 256-token output slice (fp32).
  - Host: concatenate the 8 [256, 2048] slices.

All DRAM inputs are host-pre-arranged so every DMA is contiguous per
partition. Layout key: p = partition index.
  x16  [P, KO, T]      x16[p, ko, t]     = x[t, ko*128+p]           fp16
  x32  [P, TI, KO, P]  x32[p, ti, ko, u] = x[ti*128+u, ko*128+p]    fp32
  wg16 [FI, P, KO, P]  wg16[fi, p, ko, j] = wg[ko*128+p, fi*128+j]  fp16
  wu16 likewise
  wd16 [HJ, P, FI, NH] wd16[hj, p, fo, m] = wd[fo*128+p, hj*NH+m]   fp16
  gw   [P, KO, E]      gw[p, ko, e]      = gate_w[ko*128+p, e]      fp32
  oneh [P, E]          one-hot row for this core's expert           fp32
"""

import numpy as np

H = 2048
F = 1408
E = 8
T = 2048
P = 128
KO = H // P          # 16 contraction chunks over H
FI = F // P          # 11 chunks over F
TI = T // P          # 16 token chunks of 128
NT = 512             # GEMM1 moving free dim (tokens)
TJ = T // NT         # 4
NH = 512             # GEMM2 moving free dim (hidden)
HJ = H // NH         # 4
NCORES = 8
TSL = T // NCORES    # 256-token output slice per core

_CACHE = {}


def _build_nc():
    import concourse.bacc as bacc
    import concourse.tile as tile
    import concourse.mybir as mybir

    dt = mybir.dt
    AF = mybir.ActivationFunctionType
    ALU = mybir.AluOpType

    nc = bacc.Bacc("TRN2", target_bir_lowering=False, debug=False,
                   num_devices=NCORES)

    x16 = nc.dram_tensor("x16", [P, KO, T], dt.float16, kind="ExternalInput").ap()
    x32 = nc.dram_tensor("x32", [P, TI, KO, P], dt.float32, kind="ExternalInput").ap()
    wg16 = nc.dram_tensor("wg16", [FI, P, KO, P], dt.float16, kind="ExternalInput").ap()
    wu16 = nc.dram_tensor("wu16", [FI, P, KO, P], dt.float16, kind="ExternalInput").ap()
    wd16 = nc.dram_tensor("wd16", [HJ, P, FI, NH], dt.float16, kind="ExternalInput").ap()
    gw = nc.dram_tensor("gw", [P, KO, E], dt.float32, kind="ExternalInput").ap()
    oneh = nc.dram_tensor("oneh", [P, E], dt.float32, kind="ExternalInput").ap()
    out = nc.dram_tensor("out", [TSL, H], dt.float32, kind="ExternalOutput").ap()

    with tile.TileContext(nc) as tc:
        with (
            tc.tile_pool(name="persist", bufs=1) as persist,
            tc.tile_pool(name="wpool", bufs=2) as wpool,
            tc.tile_pool(name="wdpool", bufs=2) as wdpool,
            tc.tile_pool(name="rpool", bufs=2) as rpool,
            tc.tile_pool(name="small", bufs=2) as small,
            tc.tile_pool(name="ypool", bufs=3) as ypool,
            tc.tile_pool(name="blkpool", bufs=3) as blkpool,
            tc.tile_pool(name="accpool", bufs=2) as accpool,
            tc.tile_pool(name="ps_r", bufs=2, space="PSUM") as ps_r,
            tc.tile_pool(name="ps_g", bufs=2, space="PSUM") as ps_g,
            tc.tile_pool(name="ps_u", bufs=2, space="PSUM") as ps_u,
            tc.tile_pool(name="ps_y", bufs=2, space="PSUM") as ps_y,
            tc.tile_pool(name="dram", bufs=1, space="DRAM") as dram,
        ):
            # ---- persistent SBUF tensors ----
            xsb = persist.tile([P, KO, T], dt.float16)           # 64 KB/p
            asb = persist.tile([P, FI, T], dt.float16)           # 44 KB/p
            combine = persist.tile([P, TI], dt.float32)
            gwsb = persist.tile([P, KO, E], dt.float32)
            onehsb = persist.tile([P, E], dt.float32)

            nc.sync.dma_start(gwsb[:], gw)
            nc.sync.dma_start(onehsb[:], oneh)
            for ko in range(KO):
                nc.sync.dma_start(xsb[:, ko, :], x16[:, ko, :])

            # ---- router (fp32, replicated on every core) ----
            for ti in range(TI):
                xr = rpool.tile([P, KO, P], dt.float32, tag="xr")
                nc.sync.dma_start(xr[:], x32[:, ti])
                pr = ps_r.tile([P, E], dt.float32, tag="pr")
                for ko in range(KO):
                    nc.tensor.matmul(pr[:], xr[:, ko, :], gwsb[:, ko, :],
                                     start=(ko == 0), stop=(ko == KO - 1))
                # top-2 + renormalize:
                #   m1 = max_e l;  e = exp(l - m1);  m2v = max of e excl. top1
                #   w_e = e * [e >= m2v] / (1 + m2v);  combine = w . onehot
                m1 = small.tile([P, 1], dt.float32, tag="m1")
                nc.vector.reduce_max(m1[:], pr[:], axis=mybir.AxisListType.X)
                nm1 = small.tile([P, 1], dt.float32, tag="nm1")
                nc.vector.tensor_scalar_mul(nm1[:], m1[:], -1.0)
                esb = small.tile([P, E], dt.float32, tag="esb")
                nc.scalar.activation(esb[:], pr[:], AF.Exp, bias=nm1[:])
                mask1 = small.tile([P, E], dt.float32, tag="mask1")
                nc.vector.tensor_scalar(mask1[:], pr[:], m1[:], None, op0=ALU.is_ge)
                e2 = small.tile([P, E], dt.float32, tag="e2")
                nc.vector.tensor_sub(e2[:], esb[:], mask1[:])
                m2v = small.tile([P, 1], dt.float32, tag="m2v")
                nc.vector.reduce_max(m2v[:], e2[:], axis=mybir.AxisListType.X)
                denom = small.tile([P, 1], dt.float32, tag="denom")
                nc.vector.tensor_scalar_add(denom[:], m2v[:], 1.0)
                rec = small.tile([P, 1], dt.float32, tag="rec")
                nc.vector.reciprocal(rec[:], denom[:])
                selm = small.tile([P, E], dt.float32, tag="selm")
                nc.vector.tensor_scalar(selm[:], esb[:], m2v[:], None, op0=ALU.is_ge)
                wts = small.tile([P, E], dt.float32, tag="wts")
                nc.vector.tensor_mul(wts[:], esb[:], selm[:])
                nc.vector.tensor_scalar_mul(wts[:], wts[:], rec[:])
                nc.vector.tensor_mul(wts[:], wts[:], onehsb[:])
                nc.vector.reduce_sum(combine[:, ti:ti + 1], wts[:],
                                     axis=mybir.AxisListType.X)

            # ---- GEMM1: A = silu(x @ wg) * (x @ wu), layout [F, T] fp16 ----
            for fi in range(FI):
                wgt = wpool.tile([P, KO, P], dt.float16, tag="wgt")
                nc.sync.dma_start(wgt[:], wg16[fi])
                wut = wpool.tile([P, KO, P], dt.float16, tag="wut")
                nc.sync.dma_start(wut[:], wu16[fi])
                for tj in range(TJ):
                    pg = ps_g.tile([P, NT], dt.float32, tag="pg")
                    for ko in range(KO):
                        nc.tensor.matmul(pg[:], wgt[:, ko, :],
                                         xsb[:, ko, tj * NT:(tj + 1) * NT],
                                         start=(ko == 0), stop=(ko == KO - 1))
                    pu = ps_u.tile([P, NT], dt.float32, tag="pu")
                    for ko in range(KO):
                        nc.tensor.matmul(pu[:], wut[:, ko, :],
                                         xsb[:, ko, tj * NT:(tj + 1) * NT],
                                         start=(ko == 0), stop=(ko == KO - 1))
                    a_sl = asb[:, fi, tj * NT:(tj + 1) * NT]
                    nc.scalar.activation(a_sl, pg[:], AF.Silu)
                    nc.vector.tensor_mul(a_sl, a_sl, pu[:])

            # ---- GEMM2 + scale + AllToAll staging ----
            a2a_in = dram.tile([NCORES, TSL, H], dt.float16)
            a2a_out = dram.tile([NCORES, TSL, H], dt.float16)
            for hj in range(HJ):
                wdt = wdpool.tile([P, FI, NH], dt.float16, tag="wdt")
                nc.sync.dma_start(wdt[:], wd16[hj])
                for ti in range(TI):
                    py = ps_y.tile([P, NH], dt.float32, tag="py")
                    for fi in range(FI):
                        nc.tensor.matmul(py[:], asb[:, fi, ti * P:(ti + 1) * P],
                                         wdt[:, fi, :],
                                         start=(fi == 0), stop=(fi == FI - 1))
                    y16 = ypool.tile([P, NH], dt.float16, tag="y16")
                    nc.vector.tensor_scalar_mul(y16[:], py[:], combine[:, ti:ti + 1])
                    dest = ti // (TSL // P)
                    row = (ti % (TSL // P)) * P
                    nc.sync.dma_start(
                        a2a_in[dest, row:row + P, hj * NH:(hj + 1) * NH], y16[:])

            # ---- AllToAll over token slices (fp16) ----
            nc.gpsimd.collective_compute(
                "AllToAll",
                mybir.AluOpType.bypass,
                replica_groups=[list(range(NCORES))],
                ins=[a2a_in[:].opt()],
                outs=[a2a_out[:].opt()],
            )

            # ---- sum the 8 partials for my 256-token slice ----
            for half in range(TSL // P):
                acc = accpool.tile([P, H], dt.float32, tag="acc")
                for c in range(NCORES):
                    blk = blkpool.tile([P, H], dt.float16, tag="blk")
                    nc.sync.dma_start(blk[:], a2a_out[c, half * P:(half + 1) * P, :])
                    if c == 0:
                        nc.vector.tensor_copy(acc[:], blk[:])
                    else:
                        nc.vector.tensor_add(acc[:], acc[:], blk[:])
                nc.sync.dma_start(out[half * P:(half + 1) * P, :], acc[:])

    nc.compile()
    return nc


def _get_nc():
    if "nc" not in _CACHE:
        _CACHE["nc"] = _build_nc()
    return _CACHE["nc"]


def _prep_in_maps(hidden_states, gate_w, w_gate, w_up, w_down):
    x = np.ascontiguousarray(np.asarray(hidden_states, dtype=np.float32).reshape(T, H))
    gate_w = np.asarray(gate_w, dtype=np.float32)
    w_gate = np.asarray(w_gate, dtype=np.float32)
    w_up = np.asarray(w_up, dtype=np.float32)
    w_down = np.asarray(w_down, dtype=np.float32)

    xT = x.T                                                   # [H, T]
    x16 = np.ascontiguousarray(
        xT.reshape(KO, P, T).transpose(1, 0, 2)).astype(np.float16)
    x32 = np.ascontiguousarray(
        x.reshape(TI, P, KO, P).transpose(3, 0, 2, 1))         # [P, TI, KO, P]
    gw = np.ascontiguousarray(gate_w.reshape(KO, P, E).transpose(1, 0, 2))

    in_maps = []
    for c in range(NCORES):
        wg16 = np.ascontiguousarray(
            w_gate[c].reshape(KO, P, FI, P).transpose(2, 1, 0, 3)).astype(np.float16)
        wu16 = np.ascontiguousarray(
            w_up[c].reshape(KO, P, FI, P).transpose(2, 1, 0, 3)).astype(np.float16)
        wd16 = np.ascontiguousarray(
            w_down[c].reshape(FI, P, HJ, NH).transpose(2, 1, 0, 3)).astype(np.float16)
        oneh = np.zeros((P, E), dtype=np.float32)
        oneh[:, c] = 1.0
        in_maps.append({
            "x16": x16, "x32": x32, "wg16": wg16, "wu16": wu16,
            "wd16": wd16, "gw": gw, "oneh": oneh,
        })
    return in_maps


def _run(inputs, trace=False, trace_cores=None):
    from concourse import bass_utils
    nc = _get_nc()
    in_maps = _prep_in_maps(**inputs)
    res = bass_utils.run_bass_kernel_spmd(
        nc, in_maps, core_ids=list(range(NCORES)), trace=trace,
        trace_cores=trace_cores)
    full = np.concatenate([res.results[c]["out"] for c in range(NCORES)],
                          axis=0).reshape(1, T, H).astype(np.float32)
    return full, res


def kernel(hidden_states, gate_w, w_gate, w_up, w_down):
    full, _ = _run(dict(hidden_states=hidden_states, gate_w=gate_w,
                        w_gate=w_gate, w_up=w_up, w_down=w_down))
    return full


# revision 3
# speedup vs baseline: 1.5088x; 1.5088x over previous
"""DeepSeekV2 MoE (E=8, top-2, H=2048, F=1408, T=2048) on 8 TRN2 NeuronCores.

Strategy (expert-parallel, dense):
  - Host: replicate x (transposed layouts) to all cores; shard the 8 experts'
    weight stacks one expert per core; all weights/activations for the big
    GEMMs are cast to fp16 (router stays fp32).
  - Each core c:
      router:  logits = x @ gate_w in fp32 on PE, top-2 + renormalized
               combine weights computed on-device (replicated); core c keeps
               its own expert's column via a one-hot input.
      expert:  G = wg_c.T-layout GEMM, U likewise -> A = silu(G)*U  [F, T] fp16
               Y = A.T @ wd_c  [T, H], scaled rows by combine_c      (all PE fp16)
      combine: scaled Y (fp16) AllToAll'd over the token dim; each core sums
               the 8 received partials for its 256-token output slice (fp32).
  - Host: concatenate the 8 [256, 2048] slices.

All DRAM inputs are host-pre-arranged so every DMA is contiguous per
partition. Layout key: p = partition index.
  x16  [P, KO, T]      x16[p, ko, t]     = x[t, ko*128+p]           fp16
  x32  [P, TI, KO, P]  x32[p, ti, ko, u] = x[ti*128+u, ko*128+p]    fp32
  wg16 [FI, P, KO, P]  wg16[fi, p, ko, j] = wg[ko*128+p, fi*128+j]  fp16
  wu16 likewise
  wd16 [HJ, P, FI, NH] wd16[hj, p, fo, m] = wd[fo*128+p, hj*NH+m]   fp16
  gw   [P, KO, E]      gw[p, ko, e]      = gate_w[ko*128+p, e]      fp32
  oneh [P, E]          one-hot row for this core's expert           fp32
"""

import numpy as np

H = 2048
F = 1408
E = 8
T = 2048
P = 128
KO = H // P          # 16 contraction chunks over H
FI = F // P          # 11 chunks over F
TI = T // P          # 16 token chunks of 128
NT = 512             # GEMM1 moving free dim (tokens)
TJ = T // NT         # 4
NH = 512             # GEMM2 moving free dim (hidden)
HJ = H // NH         # 4
NCORES = 8
TSL = T // NCORES    # 256-token output slice per core

_CACHE = {}


def _build_nc():
    import concourse.bacc as bacc
    import concourse.tile as tile
    import concourse.mybir as mybir

    dt = mybir.dt
    AF = mybir.ActivationFunctionType
    ALU = mybir.AluOpType

    nc = bacc.Bacc("TRN2", target_bir_lowering=False, debug=False,
                   num_devices=NCORES)

    x16 = nc.dram_tensor("x16", [P, KO, T], dt.float16, kind="ExternalInput").ap()
    x32 = nc.dram_tensor("x32", [P, TI, KO, P], dt.float32, kind="ExternalInput").ap()
    wg16 = nc.dram_tensor("wg16", [FI, P, KO, P], dt.float16, kind="ExternalInput").ap()
    wu16 = nc.dram_tensor("wu16", [FI, P, KO, P], dt.float16, kind="ExternalInput").ap()
    wd16 = nc.dram_tensor("wd16", [HJ, P, FI, NH], dt.float16, kind="ExternalInput").ap()
    gw = nc.dram_tensor("gw", [P, KO, E], dt.float32, kind="ExternalInput").ap()
    oneh = nc.dram_tensor("oneh", [P, E], dt.float32, kind="ExternalInput").ap()
    out = nc.dram_tensor("out", [TSL, H], dt.float32, kind="ExternalOutput").ap()

    with tile.TileContext(nc) as tc:
        with (
            tc.tile_pool(name="persist", bufs=1) as persist,
            tc.tile_pool(name="wpool", bufs=2) as wpool,
            tc.tile_pool(name="wdpool", bufs=2) as wdpool,
            tc.tile_pool(name="rpool", bufs=2) as rpool,
            tc.tile_pool(name="small", bufs=2) as small,
            tc.tile_pool(name="ypool", bufs=3) as ypool,
            tc.tile_pool(name="blkpool", bufs=3) as blkpool,
            tc.tile_pool(name="accpool", bufs=2) as accpool,
            tc.tile_pool(name="ps_r", bufs=2, space="PSUM") as ps_r,
            tc.tile_pool(name="ps_g", bufs=2, space="PSUM") as ps_g,
            tc.tile_pool(name="ps_u", bufs=2, space="PSUM") as ps_u,
            tc.tile_pool(name="ps_y", bufs=2, space="PSUM") as ps_y,
            tc.tile_pool(name="dram", bufs=1, space="DRAM") as dram,
        ):
            # ---- persistent SBUF tensors ----
            xsb = persist.tile([P, KO, T], dt.float16)           # 64 KB/p
            asb = persist.tile([P, FI, T], dt.float16)           # 44 KB/p
            combine = persist.tile([P, TI], dt.float32)
            gwsb = persist.tile([P, KO, E], dt.float32)
            onehsb = persist.tile([P, E], dt.float32)

            nc.sync.dma_start(gwsb[:], gw)
            nc.sync.dma_start(onehsb[:], oneh)
            for ko in range(KO):
                nc.sync.dma_start(xsb[:, ko, :], x16[:, ko, :])

            # ---- router (fp32, replicated on every core) ----
            for ti in range(TI):
                xr = rpool.tile([P, KO, P], dt.float32, tag="xr")
                nc.sync.dma_start(xr[:], x32[:, ti])
                pr = ps_r.tile([P, E], dt.float32, tag="pr")
                for ko in range(KO):
                    nc.tensor.matmul(pr[:], xr[:, ko, :], gwsb[:, ko, :],
                                     start=(ko == 0), stop=(ko == KO - 1))
                # top-2 + renormalize:
                #   m1 = max_e l;  e = exp(l - m1);  m2v = max of e excl. top1
                #   w_e = e * [e >= m2v] / (1 + m2v);  combine = w . onehot
                m1 = small.tile([P, 1], dt.float32, tag="m1")
                nc.vector.reduce_max(m1[:], pr[:], axis=mybir.AxisListType.X)
                nm1 = small.tile([P, 1], dt.float32, tag="nm1")
                nc.vector.tensor_scalar_mul(nm1[:], m1[:], -1.0)
                esb = small.tile([P, E], dt.float32, tag="esb")
                nc.scalar.activation(esb[:], pr[:], AF.Exp, bias=nm1[:])
                mask1 = small.tile([P, E], dt.float32, tag="mask1")
                nc.vector.tensor_scalar(mask1[:], pr[:], m1[:], None, op0=ALU.is_ge)
                e2 = small.tile([P, E], dt.float32, tag="e2")
                nc.vector.tensor_sub(e2[:], esb[:], mask1[:])
                m2v = small.tile([P, 1], dt.float32, tag="m2v")
                nc.vector.reduce_max(m2v[:], e2[:], axis=mybir.AxisListType.X)
                denom = small.tile([P, 1], dt.float32, tag="denom")
                nc.vector.tensor_scalar_add(denom[:], m2v[:], 1.0)
                rec = small.tile([P, 1], dt.float32, tag="rec")
                nc.vector.reciprocal(rec[:], denom[:])
                selm = small.tile([P, E], dt.float32, tag="selm")
                nc.vector.tensor_scalar(selm[:], esb[:], m2v[:], None, op0=ALU.is_ge)
                wts = small.tile([P, E], dt.float32, tag="wts")
                nc.vector.tensor_mul(wts[:], esb[:], selm[:])
                nc.vector.tensor_scalar_mul(wts[:], wts[:], rec[:])
                nc.vector.tensor_mul(wts[:], wts[:], onehsb[:])
                nc.vector.reduce_sum(combine[:, ti:ti + 1], wts[:],
                                     axis=mybir.AxisListType.X)

            # ---- GEMM1: A = silu(x @ wg) * (x @ wu), layout [F, T] fp16 ----
            for fi in range(FI):
                wgt = wpool.tile([P, KO, P], dt.float16, tag="wgt")
                nc.sync.dma_start(wgt[:], wg16[fi])
                wut = wpool.tile([P, KO, P], dt.float16, tag="wut")
                nc.sync.dma_start(wut[:], wu16[fi])
                for tj in range(TJ):
                    pg = ps_g.tile([P, NT], dt.float32, tag="pg")
                    for ko in range(KO):
                        nc.tensor.matmul(pg[:], wgt[:, ko, :],
                                         xsb[:, ko, tj * NT:(tj + 1) * NT],
                                         start=(ko == 0), stop=(ko == KO - 1))
                    pu = ps_u.tile([P, NT], dt.float32, tag="pu")
                    for ko in range(KO):
                        nc.tensor.matmul(pu[:], wut[:, ko, :],
                                         xsb[:, ko, tj * NT:(tj + 1) * NT],
                                         start=(ko == 0), stop=(ko == KO - 1))
                    a_sl = asb[:, fi, tj * NT:(tj + 1) * NT]
                    nc.scalar.activation(a_sl, pg[:], AF.Silu)
                    nc.vector.tensor_mul(a_sl, a_sl, pu[:])

            # ---- GEMM2 + scale + chunked AllToAll (one per h-block) ----
            a2a_ins = []
            a2a_outs = []
            for hj in range(HJ):
                a_in = dram.tile([NCORES, TSL, NH], dt.float16, name=f"a2a_in{hj}")
                a_out = dram.tile([NCORES, TSL, NH], dt.float16, name=f"a2a_out{hj}")
                a2a_ins.append(a_in)
                a2a_outs.append(a_out)
            for hj in range(HJ):
                wdt = wdpool.tile([P, FI, NH], dt.float16, tag="wdt")
                nc.sync.dma_start(wdt[:], wd16[hj])
                for ti in range(TI):
                    py = ps_y.tile([P, NH], dt.float32, tag="py")
                    for fi in range(FI):
                        nc.tensor.matmul(py[:], asb[:, fi, ti * P:(ti + 1) * P],
                                         wdt[:, fi, :],
                                         start=(fi == 0), stop=(fi == FI - 1))
                    y16 = ypool.tile([P, NH], dt.float16, tag="y16")
                    nc.vector.tensor_scalar_mul(y16[:], py[:], combine[:, ti:ti + 1])
                    dest = ti // (TSL // P)
                    row = (ti % (TSL // P)) * P
                    nc.sync.dma_start(a2a_ins[hj][dest, row:row + P, :], y16[:])
                nc.gpsimd.collective_compute(
                    "AllToAll",
                    mybir.AluOpType.bypass,
                    replica_groups=[list(range(NCORES))],
                    ins=[a2a_ins[hj][:].opt()],
                    outs=[a2a_outs[hj][:].opt()],
                )
                # sum the 8 partials for my 256-token slice of this h-block
                for half in range(TSL // P):
                    acc = accpool.tile([P, NH], dt.float32, tag="acc")
                    for c in range(NCORES):
                        blk = blkpool.tile([P, NH], dt.float16, tag="blk")
                        nc.sync.dma_start(blk[:],
                                          a2a_outs[hj][c, half * P:(half + 1) * P, :])
                        if c == 0:
                            nc.vector.tensor_copy(acc[:], blk[:])
                        else:
                            nc.vector.tensor_add(acc[:], acc[:], blk[:])
                    nc.sync.dma_start(
                        out[half * P:(half + 1) * P, hj * NH:(hj + 1) * NH], acc[:])

    nc.compile()
    return nc


def _get_nc():
    if "nc" not in _CACHE:
        _CACHE["nc"] = _build_nc()
    return _CACHE["nc"]


def _prep_in_maps(hidden_states, gate_w, w_gate, w_up, w_down):
    x = np.ascontiguousarray(np.asarray(hidden_states, dtype=np.float32).reshape(T, H))
    gate_w = np.asarray(gate_w, dtype=np.float32)
    w_gate = np.asarray(w_gate, dtype=np.float32)
    w_up = np.asarray(w_up, dtype=np.float32)
    w_down = np.asarray(w_down, dtype=np.float32)

    xT = x.T                                                   # [H, T]
    x16 = np.ascontiguousarray(
        xT.reshape(KO, P, T).transpose(1, 0, 2)).astype(np.float16)
    x32 = np.ascontiguousarray(
        x.reshape(TI, P, KO, P).transpose(3, 0, 2, 1))         # [P, TI, KO, P]
    gw = np.ascontiguousarray(gate_w.reshape(KO, P, E).transpose(1, 0, 2))

    in_maps = []
    for c in range(NCORES):
        wg16 = np.ascontiguousarray(
            w_gate[c].reshape(KO, P, FI, P).transpose(2, 1, 0, 3)).astype(np.float16)
        wu16 = np.ascontiguousarray(
            w_up[c].reshape(KO, P, FI, P).transpose(2, 1, 0, 3)).astype(np.float16)
        wd16 = np.ascontiguousarray(
            w_down[c].reshape(FI, P, HJ, NH).transpose(2, 1, 0, 3)).astype(np.float16)
        oneh = np.zeros((P, E), dtype=np.float32)
        oneh[:, c] = 1.0
        in_maps.append({
            "x16": x16, "x32": x32, "wg16": wg16, "wu16": wu16,
            "wd16": wd16, "gw": gw, "oneh": oneh,
        })
    return in_maps


def _run(inputs, trace=False, trace_cores=None):
    from concourse import bass_utils
    nc = _get_nc()
    in_maps = _prep_in_maps(**inputs)
    res = bass_utils.run_bass_kernel_spmd(
        nc, in_maps, core_ids=list(range(NCORES)), trace=trace,
        trace_cores=trace_cores)
    full = np.concatenate([res.results[c]["out"] for c in range(NCORES)],
                          axis=0).reshape(1, T, H).astype(np.float32)
    return full, res


def kernel(hidden_states, gate_w, w_gate, w_up, w_down):
    full, _ = _run(dict(hidden_states=hidden_states, gate_w=gate_w,
                        w_gate=w_gate, w_up=w_up, w_down=w_down))
    return full
